# revision 6
# baseline (speedup 1.0000x reference)
"""Trainium2 Bass kernel: MultiHeadAttention over [2, 512, 64, 64] images.

Sharding: 8 cores = (2 batches) x (4 head-pairs). Each core computes 2 of the
8 attention heads for one batch plus a partial output projection over its 128
input channels; the host sums the 4 partial projections per batch (the unshard
step for a contraction-dim tensor-parallel split).

Per-core pipeline (all L=4096 positions):
  QKV:  Q/K in [c=128, l] layout (2 heads x 64 dk-channels on partitions),
        V transposed on the PE into VT [s, c] with an interleaved ones column.
  Attn: S^T tiles [s=128, t=512] via row-packed K=64 matmuls (2 heads share
        the PE array), Exp on the scalar engine straight out of PSUM (scores
        are O(+-1.4) so softmax needs no max subtraction), AV matmuls with a
        65th ones-row so the softmax denominator accumulates in PSUM row 64,
        normalization via DVE reciprocal + partition broadcast.
  Proj: partial Wp projection of the normalized heads, bias on one core/batch.

Matmul operands use float32r (~1e-4 rel err, 4x the fp32 matmul rate).
"""

import math
import numpy as np

B, C, HH, WW = 2, 512, 64, 64
L = HH * WW          # 4096
NH, DK = 8, 64
SCALE = 1.0 / math.sqrt(DK)
NCORES = 8

TT = 512             # t-tile width (columns per attention tile)
NT = L // TT         # 8 t-tiles
NS = L // 128        # 32 s-tiles
KT = C // 128        # 4 contraction tiles for projections

_BUILT = {}


def _build(l=L):
    import concourse.bacc as bacc
    import concourse.tile as tile
    import concourse.mybir as mybir
    import concourse.bass as bass
    from concourse.masks import make_identity
    from contextlib import ExitStack

    nt = l // TT
    ns = l // 128
    f32 = mybir.dt.float32
    f16 = mybir.dt.float16
    f32r = mybir.dt.float32r
    Exp = mybir.ActivationFunctionType.Exp
    add = mybir.AluOpType.add

    nc = bacc.Bacc("TRN2", target_bir_lowering=False, debug=False,
                   num_devices=NCORES)

    x = nc.dram_tensor("x", [C, l], f16, kind="ExternalInput").ap()
    wq = nc.dram_tensor("wq", [C, 128], f16, kind="ExternalInput").ap()
    wk = nc.dram_tensor("wk", [C, 128], f16, kind="ExternalInput").ap()
    wv = nc.dram_tensor("wv", [C, 128], f16, kind="ExternalInput").ap()
    bq = nc.dram_tensor("bq", [128, 1], f32, kind="ExternalInput").ap()
    bk = nc.dram_tensor("bk", [128, 1], f32, kind="ExternalInput").ap()
    bv = nc.dram_tensor("bv", [128, 1], f32, kind="ExternalInput").ap()
    wp = nc.dram_tensor("wp", [128, C], f16, kind="ExternalInput").ap()
    bp = nc.dram_tensor("bp", [128, KT], f32, kind="ExternalInput").ap()
    out = nc.dram_tensor("out", [C, l], f32, kind="ExternalOutput").ap()

    with tile.TileContext(nc) as tc, ExitStack() as ctx:
        persist = ctx.enter_context(tc.tile_pool(name="persist", bufs=1))
        e_pool = ctx.enter_context(tc.tile_pool(name="e", bufs=4))
        o_pool = ctx.enter_context(tc.tile_pool(name="o", bufs=2))
        z_pool = ctx.enter_context(tc.tile_pool(name="z", bufs=2))
        res_pool = ctx.enter_context(tc.tile_pool(name="res", bufs=3))

        # ---- weights: fp16 straight from DRAM ----
        w_r = persist.tile([128, 3, KT, 128], f16, tag="wr")
        for i, w in enumerate((wq, wk, wv)):
            for kt in range(KT):
                nc.sync.dma_start(out=w_r[:, i, kt, :],
                                  in_=w[kt * 128:(kt + 1) * 128, :])
        wp_r = persist.tile([128, C], f16, tag="wpr")
        nc.sync.dma_start(out=wp_r, in_=wp)

        bias_sb = persist.tile([128, 3], f32, tag="bias")
        for i, bvec in enumerate((bq, bk, bv)):
            nc.sync.dma_start(out=bias_sb[:, i:i + 1], in_=bvec)
        bp_sb = persist.tile([128, KT], f32, tag="bp")
        nc.sync.dma_start(out=bp_sb, in_=bp)

        ident = persist.tile([128, 128], f16, tag="ident")
        make_identity(nc, ident)

        ones_sb = persist.tile([1, 64], f32, tag="ones")
        nc.vector.memset(ones_sb, 1.0)
        ones_r = persist.tile([1, 64], f32r, tag="onesr")
        nc.vector.tensor_copy(ones_r, ones_sb)

        # ---- persistent activations ----
        q_sb = persist.tile([128, l], f16, tag="q")
        k_sb = persist.tile([128, l], f16, tag="k")
        vt_sb = persist.tile([128, ns, 130], f16, tag="vt")
        # interleaved ones columns -> softmax denominator rows in AV psum
        ones_col = persist.tile([128, ns, 1], f16, tag="onescol")
        nc.vector.memset(ones_col, 1.0)
        nc.vector.tensor_copy(vt_sb[:, :, 64:65], ones_col)
        nc.vector.tensor_copy(vt_sb[:, :, 129:130], ones_col)

        # ================= QKV projections + V transpose =================
        with ExitStack() as qctx:
            x_pool = qctx.enter_context(tc.tile_pool(name="xs", bufs=3))
            qkv_ps = qctx.enter_context(
                tc.tile_pool(name="qkvps", bufs=2, space="PSUM"))
            v_pool = qctx.enter_context(tc.tile_pool(name="vsb", bufs=2))

            for n in range(nt):
                nsl = slice(n * TT, (n + 1) * TT)
                x_r = x_pool.tile([128, KT, TT], f16, tag="x")
                for kt in range(KT):
                    nc.sync.dma_start(out=x_r[:, kt, :],
                                      in_=x[kt * 128:(kt + 1) * 128, nsl])

                q_ps = qkv_ps.tile([128, TT], f32, tag="qps")
                k_ps = qkv_ps.tile([128, TT], f32, tag="kps")
                v_ps = qkv_ps.tile([128, TT], f32, tag="vps")
                for kt in range(KT):
                    st, sp = (kt == 0), (kt == KT - 1)
                    nc.tensor.matmul(q_ps, w_r[:, 0, kt, :], x_r[:, kt, :],
                                     start=st, stop=sp)
                    nc.tensor.matmul(k_ps, w_r[:, 1, kt, :], x_r[:, kt, :],
                                     start=st, stop=sp)
                    nc.tensor.matmul(v_ps, w_r[:, 2, kt, :], x_r[:, kt, :],
                                     start=st, stop=sp)

                nc.vector.tensor_scalar(q_sb[:, nsl], q_ps,
                                        bias_sb[:, 0:1], None, add)
                nc.vector.tensor_scalar(k_sb[:, nsl], k_ps,
                                        bias_sb[:, 1:2], None, add)
                v_sb = v_pool.tile([128, TT], f16, tag="v")
                nc.vector.tensor_scalar(v_sb, v_ps, bias_sb[:, 2:3], None, add)

                # transpose V tile: 4 PE transposes -> [s, c] in psum
                tp = qkv_ps.tile([128, TT], f16, tag="tp")
                for j in range(4):
                    nc.tensor.transpose(tp[:, j * 128:(j + 1) * 128],
                                        v_sb[:, j * 128:(j + 1) * 128], ident)
                tp_v = tp.rearrange("p (j c) -> p j c", j=4)
                ssl = slice(4 * n, 4 * n + 4)
                nc.vector.tensor_copy(vt_sb[:, ssl, 0:64], tp_v[:, :, 0:64])
                nc.vector.tensor_copy(vt_sb[:, ssl, 65:129], tp_v[:, :, 64:128])

        # ========================= attention =========================
        with ExitStack() as actx:
            st_pool = actx.enter_context(
                tc.tile_pool(name="stps", bufs=2, space="PSUM"))
            av_pool = actx.enter_context(
                tc.tile_pool(name="avps", bufs=3, space="PSUM"))
            pr_pool = actx.enter_context(
                tc.tile_pool(name="prps", bufs=1, space="PSUM"))

            for t in range(nt):
                tsl = slice(t * TT, (t + 1) * TT)
                av0 = av_pool.tile([128, TT], f32, tag="av")
                av1 = av_pool.tile([128, TT], f32, tag="av")

                for s in range(ns):
                    st_ps = st_pool.tile([128, 2 * TT], f32, tag="st")
                    ssl = slice(s * 128, (s + 1) * 128)
                    nc.tensor.matmul(st_ps[:, 0:TT], k_sb[0:64, ssl],
                                     q_sb[0:64, tsl], start=True, stop=True)
                    nc.tensor.matmul(st_ps[:, TT:2 * TT], k_sb[64:128, ssl],
                                     q_sb[64:128, tsl], start=True, stop=True)
                    e_sb = e_pool.tile([128, 2 * TT], f16, tag="e")
                    nc.scalar.activation(e_sb, st_ps, Exp, scale=SCALE)
                    st, sp = (s == 0), (s == ns - 1)
                    nc.tensor.matmul(av0[0:65, :], vt_sb[:, s, 0:65],
                                     e_sb[:, 0:TT], start=st, stop=sp)
                    nc.tensor.matmul(av1[0:65, :], vt_sb[:, s, 65:130],
                                     e_sb[:, TT:2 * TT], start=st, stop=sp)

                # softmax denominators: psum row 64 -> sbuf rows, then a PE
                # outer product (ones x zrow) broadcasts each across 64
                # partitions; reciprocal + multiply normalize both heads.
                zr0 = z_pool.tile([1, TT], f32r, tag="zr0")
                zr1 = z_pool.tile([1, TT], f32r, tag="zr1")
                nc.vector.tensor_copy(zr0, av0[64:65, :])
                nc.vector.tensor_copy(zr1, av1[64:65, :])
                rz = z_pool.tile([128, TT], f32, tag="rz")
                zb0 = pr_pool.tile([64, TT], f32, tag="pp")
                nc.tensor.matmul(zb0, ones_r, zr0, start=True, stop=True)
                nc.vector.reciprocal(rz[0:64, :], zb0)
                zb1 = pr_pool.tile([64, TT], f32, tag="pp")
                nc.tensor.matmul(zb1, ones_r, zr1, start=True, stop=True)
                nc.vector.reciprocal(rz[64:128, :], zb1)

                o_sb = o_pool.tile([128, TT], f16, tag="o")
                nc.vector.tensor_mul(o_sb[0:64, :], av0[0:64, :], rz[0:64, :])
                nc.vector.tensor_mul(o_sb[64:128, :], av1[0:64, :],
                                     rz[64:128, :])

                # partial output projection over this core's 128 channels
                for ot in range(KT):
                    pp = pr_pool.tile([128, TT], f32, tag="pp")
                    nc.tensor.matmul(pp, wp_r[:, ot * 128:(ot + 1) * 128],
                                     o_sb, start=True, stop=True)
                    res = res_pool.tile([128, TT], f32, tag="res")
                    nc.vector.tensor_scalar(res, pp, bp_sb[:, ot:ot + 1],
                                            None, add)
                    nc.sync.dma_start(out=out[ot * 128:(ot + 1) * 128, tsl],
                                      in_=res)

    nc.compile()
    return nc


def _get_nc(l=L):
    if l not in _BUILT:
        _BUILT[l] = _build(l)
    return _BUILT[l]


def _shard_inputs(x, Wq, bq, Wkv, bkv, Wp, bp, l=L):
    x = np.asarray(x, dtype=np.float32)
    Wq = np.asarray(Wq, dtype=np.float32)
    bq = np.asarray(bq, dtype=np.float32)
    Wkv = np.asarray(Wkv, dtype=np.float32)
    bkv = np.asarray(bkv, dtype=np.float32)
    Wp = np.asarray(Wp, dtype=np.float32)
    bp = np.asarray(bp, dtype=np.float32)

    in_maps = []
    for core in range(NCORES):
        b, hp = divmod(core, 4)
        sl = slice(hp * 128, (hp + 1) * 128)
        vsl = slice(C + hp * 128, C + (hp + 1) * 128)
        m = {
            "x": np.ascontiguousarray(x[b].reshape(C, l).astype(np.float16)),
            "wq": np.ascontiguousarray(Wq[sl, :].T.astype(np.float16)),
            "bq": np.ascontiguousarray(bq[sl].reshape(128, 1)),
            "wk": np.ascontiguousarray(Wkv[sl, :].T.astype(np.float16)),
            "bk": np.ascontiguousarray(bkv[sl].reshape(128, 1)),
            "wv": np.ascontiguousarray(Wkv[vsl, :].T.astype(np.float16)),
            "bv": np.ascontiguousarray(bkv[vsl].reshape(128, 1)),
            "wp": np.ascontiguousarray(Wp[:, sl].T.astype(np.float16)),
            "bp": np.ascontiguousarray(
                (bp if hp == 0 else np.zeros_like(bp)).reshape(KT, 128).T),
        }
        in_maps.append(m)
    return in_maps


def _run(in_maps, l=L, trace=False):
    from concourse.bass_utils import run_bass_kernel_spmd
    nc = _get_nc(l)
    return run_bass_kernel_spmd(nc, in_maps, core_ids=list(range(NCORES)),
                                trace=trace)


def kernel(x, Wq, bq, Wkv, bkv, Wp, bp):
    in_maps = _shard_inputs(x, Wq, bq, Wkv, bkv, Wp, bp)
    res = _run(in_maps)
    outs = [res.results[i]["out"] for i in range(NCORES)]
    y = np.stack([outs[0] + outs[1] + outs[2] + outs[3],
                  outs[4] + outs[5] + outs[6] + outs[7]])
    return np.ascontiguousarray(y.reshape(B, C, HH, WW), dtype=np.float32)


# revision 11
# speedup vs baseline: 1.1943x; 1.1943x over previous
"""Trainium2 Bass kernel: MultiHeadAttention over [2, 512, 64, 64] images.

Sharding: 8 cores = (2 batches) x (4 head-pairs). Each core computes 2 of the
8 attention heads for one batch plus a partial output projection over its 128
input channels; the host sums the 4 partial projections per batch (the unshard
step for a contraction-dim tensor-parallel split).

Per-core pipeline (all L=4096 positions):
  QKV:  Q/K in [c=128, l] layout (2 heads x 64 dk-channels on partitions),
        V transposed on the PE into VT [s, c] with an interleaved ones column.
  Attn: S^T tiles [s=128, t=512] via row-packed K=64 matmuls (2 heads share
        the PE array), Exp on the scalar engine straight out of PSUM (scores
        are O(+-1.4) so softmax needs no max subtraction), AV matmuls with a
        65th ones-row so the softmax denominator accumulates in PSUM row 64,
        normalization via DVE reciprocal + partition broadcast.
  Proj: partial Wp projection of the normalized heads, bias on one core/batch.

Matmul operands use float32r (~1e-4 rel err, 4x the fp32 matmul rate).
"""

import math
import numpy as np

B, C, HH, WW = 2, 512, 64, 64
L = HH * WW          # 4096
NH, DK = 8, 64
SCALE = 1.0 / math.sqrt(DK)
NCORES = 8

TT = 512             # t-tile width (columns per attention tile)
NT = L // TT         # 8 t-tiles
NS = L // 128        # 32 s-tiles
KT = C // 128        # 4 contraction tiles for projections

_BUILT = {}


def _build(l=L):
    import concourse.bacc as bacc
    import concourse.tile as tile
    import concourse.mybir as mybir
    import concourse.bass as bass
    from concourse.masks import make_identity
    from contextlib import ExitStack

    nt = l // TT
    ns = l // 128
    f32 = mybir.dt.float32
    f16 = mybir.dt.float16
    f32r = mybir.dt.float32r
    Exp = mybir.ActivationFunctionType.Exp
    add = mybir.AluOpType.add

    nc = bacc.Bacc("TRN2", target_bir_lowering=False, debug=False,
                   num_devices=NCORES)

    x = nc.dram_tensor("x", [C, l], f16, kind="ExternalInput").ap()
    wq = nc.dram_tensor("wq", [C, 128], f16, kind="ExternalInput").ap()
    wk = nc.dram_tensor("wk", [C, 128], f16, kind="ExternalInput").ap()
    wv = nc.dram_tensor("wv", [C, 128], f16, kind="ExternalInput").ap()
    bq = nc.dram_tensor("bq", [128, 1], f32, kind="ExternalInput").ap()
    bk = nc.dram_tensor("bk", [128, 1], f32, kind="ExternalInput").ap()
    bv = nc.dram_tensor("bv", [128, 1], f32, kind="ExternalInput").ap()
    wp = nc.dram_tensor("wp", [128, C], f16, kind="ExternalInput").ap()
    bp = nc.dram_tensor("bp", [128, KT], f32, kind="ExternalInput").ap()
    out = nc.dram_tensor("out", [C, l], f32, kind="ExternalOutput").ap()

    with tile.TileContext(nc) as tc, ExitStack() as ctx:
        persist = ctx.enter_context(tc.tile_pool(name="persist", bufs=1))
        e_pool = ctx.enter_context(tc.tile_pool(name="e", bufs=6))
        o_pool = ctx.enter_context(tc.tile_pool(name="o", bufs=2))
        z_pool = ctx.enter_context(tc.tile_pool(name="z", bufs=2))
        res_pool = ctx.enter_context(tc.tile_pool(name="res", bufs=3))

        # ---- weights: fp16 straight from DRAM ----
        w_r = persist.tile([128, 3, KT, 128], f16, tag="wr")
        for i, w in enumerate((wq, wk, wv)):
            for kt in range(KT):
                nc.sync.dma_start(out=w_r[:, i, kt, :],
                                  in_=w[kt * 128:(kt + 1) * 128, :])
        wp_r = persist.tile([128, C], f16, tag="wpr")
        nc.sync.dma_start(out=wp_r, in_=wp)

        bias_sb = persist.tile([128, 3], f32, tag="bias")
        for i, bvec in enumerate((bq, bk, bv)):
            nc.sync.dma_start(out=bias_sb[:, i:i + 1], in_=bvec)
        bp_sb = persist.tile([128, KT], f32, tag="bp")
        nc.sync.dma_start(out=bp_sb, in_=bp)

        ident = persist.tile([128, 128], f16, tag="ident")
        make_identity(nc, ident)

        ones_sb = persist.tile([1, 64], f32, tag="ones")
        nc.vector.memset(ones_sb, 1.0)
        ones_r = persist.tile([1, 64], f32r, tag="onesr")
        nc.vector.tensor_copy(ones_r, ones_sb)

        # ---- persistent activations ----
        q_sb = persist.tile([128, l], f16, tag="q")
        k_sb = persist.tile([128, l], f16, tag="k")
        vt_sb = persist.tile([128, ns, 130], f32r, tag="vt")
        # interleaved ones columns -> softmax denominator rows in AV psum
        ones_col = persist.tile([128, ns, 1], f32, tag="onescol")
        nc.vector.memset(ones_col, 1.0)
        nc.vector.tensor_copy(vt_sb[:, :, 64:65], ones_col)
        nc.vector.tensor_copy(vt_sb[:, :, 129:130], ones_col)

        # ================= QKV projections + V transpose =================
        with ExitStack() as qctx:
            x_pool = qctx.enter_context(tc.tile_pool(name="xs", bufs=3))
            qkv_ps = qctx.enter_context(
                tc.tile_pool(name="qkvps", bufs=2, space="PSUM"))
            v_pool = qctx.enter_context(tc.tile_pool(name="vsb", bufs=2))

            for n in range(nt):
                nsl = slice(n * TT, (n + 1) * TT)
                x_r = x_pool.tile([128, KT, TT], f16, tag="x")
                for kt in range(KT):
                    nc.sync.dma_start(out=x_r[:, kt, :],
                                      in_=x[kt * 128:(kt + 1) * 128, nsl])

                q_ps = qkv_ps.tile([128, TT], f32, tag="qps")
                k_ps = qkv_ps.tile([128, TT], f32, tag="kps")
                v_ps = qkv_ps.tile([128, TT], f32, tag="vps")
                for kt in range(KT):
                    st, sp = (kt == 0), (kt == KT - 1)
                    nc.tensor.matmul(q_ps, w_r[:, 0, kt, :], x_r[:, kt, :],
                                     start=st, stop=sp)
                    nc.tensor.matmul(k_ps, w_r[:, 1, kt, :], x_r[:, kt, :],
                                     start=st, stop=sp)
                    nc.tensor.matmul(v_ps, w_r[:, 2, kt, :], x_r[:, kt, :],
                                     start=st, stop=sp)

                nc.vector.tensor_scalar(q_sb[:, nsl], q_ps,
                                        bias_sb[:, 0:1], None, add)
                nc.vector.tensor_scalar(k_sb[:, nsl], k_ps,
                                        bias_sb[:, 1:2], None, add)
                v_sb = v_pool.tile([128, TT], f16, tag="v")
                nc.vector.tensor_scalar(v_sb, v_ps, bias_sb[:, 2:3], None, add)

                # transpose V tile: 4 PE transposes -> [s, c] in psum
                tp = qkv_ps.tile([128, TT], f16, tag="tp")
                for j in range(4):
                    nc.tensor.transpose(tp[:, j * 128:(j + 1) * 128],
                                        v_sb[:, j * 128:(j + 1) * 128], ident)
                tp_v = tp.rearrange("p (j c) -> p j c", j=4)
                ssl = slice(4 * n, 4 * n + 4)
                nc.vector.tensor_copy(vt_sb[:, ssl, 0:64], tp_v[:, :, 0:64])
                nc.vector.tensor_copy(vt_sb[:, ssl, 65:129], tp_v[:, :, 64:128])

        # ========================= attention =========================
        with ExitStack() as actx:
            st_pool = actx.enter_context(
                tc.tile_pool(name="stps", bufs=2, space="PSUM"))
            av_pool = actx.enter_context(
                tc.tile_pool(name="avps", bufs=3, space="PSUM"))
            pr_pool = actx.enter_context(
                tc.tile_pool(name="prps", bufs=1, space="PSUM"))

            for t in range(nt):
                tsl = slice(t * TT, (t + 1) * TT)
                av0 = av_pool.tile([128, TT], f32, tag="av")
                av1 = av_pool.tile([128, TT], f32, tag="av")

                for s in range(ns):
                    st_ps = st_pool.tile([128, 2 * TT], f32, tag="st")
                    ssl = slice(s * 128, (s + 1) * 128)
                    nc.tensor.matmul(st_ps[:, 0:TT], k_sb[0:64, ssl],
                                     q_sb[0:64, tsl], start=True, stop=True)
                    nc.tensor.matmul(st_ps[:, TT:2 * TT], k_sb[64:128, ssl],
                                     q_sb[64:128, tsl], start=True, stop=True)
                    e_sb = e_pool.tile([128, 2 * TT], f32r, tag="e")
                    nc.scalar.activation(e_sb, st_ps, Exp, scale=SCALE)
                    st, sp = (s == 0), (s == ns - 1)
                    nc.tensor.matmul(av0[0:65, :], vt_sb[:, s, 0:65],
                                     e_sb[:, 0:TT], start=st, stop=sp)
                    nc.tensor.matmul(av1[0:65, :], vt_sb[:, s, 65:130],
                                     e_sb[:, TT:2 * TT], start=st, stop=sp)

                # Evacuate the AV psums immediately (unnormalized heads +
                # denominator rows) so their slots free fast -- a slow
                # normalize chain here stalls the in-order PE queue long
                # enough for HAM to re-throttle the clock every t-tile.
                ou = o_pool.tile([128, TT], f32, tag="ou")
                zr0 = z_pool.tile([1, TT], f32r, tag="zr0")
                nc.vector.tensor_copy(ou[0:64, :], av0[0:64, :])
                nc.vector.tensor_copy(zr0, av0[64:65, :])
                zr1 = z_pool.tile([1, TT], f32r, tag="zr1")
                nc.vector.tensor_copy(ou[64:128, :], av1[0:64, :])
                nc.vector.tensor_copy(zr1, av1[64:65, :])

                # ones (x) zrow outer products broadcast each denominator
                # across 64 partitions; fast reciprocal + multiply normalize.
                rz = z_pool.tile([128, TT], f32, tag="rz")
                zb0 = pr_pool.tile([64, TT], f32, tag="pp")
                nc.tensor.matmul(zb0, ones_r, zr0, start=True, stop=True)
                nc.vector.reciprocal(rz[0:64, :], zb0)
                zb1 = pr_pool.tile([64, TT], f32, tag="pp")
                nc.tensor.matmul(zb1, ones_r, zr1, start=True, stop=True)
                nc.vector.reciprocal(rz[64:128, :], zb1)

                o_sb = o_pool.tile([128, TT], f16, tag="o")
                nc.vector.tensor_mul(o_sb[0:64, :], ou[0:64, :], rz[0:64, :])
                nc.vector.tensor_mul(o_sb[64:128, :], ou[64:128, :],
                                     rz[64:128, :])

                # partial output projection over this core's 128 channels
                for ot in range(KT):
                    pp = pr_pool.tile([128, TT], f32, tag="pp")
                    nc.tensor.matmul(pp, wp_r[:, ot * 128:(ot + 1) * 128],
                                     o_sb, start=True, stop=True)
                    res = res_pool.tile([128, TT], f32, tag="res")
                    nc.vector.tensor_scalar(res, pp, bp_sb[:, ot:ot + 1],
                                            None, add)
                    nc.sync.dma_start(out=out[ot * 128:(ot + 1) * 128, tsl],
                                      in_=res)

    nc.compile()
    return nc


def _get_nc(l=L):
    if l not in _BUILT:
        _BUILT[l] = _build(l)
    return _BUILT[l]


def _shard_inputs(x, Wq, bq, Wkv, bkv, Wp, bp, l=L):
    x = np.asarray(x, dtype=np.float32)
    Wq = np.asarray(Wq, dtype=np.float32)
    bq = np.asarray(bq, dtype=np.float32)
    Wkv = np.asarray(Wkv, dtype=np.float32)
    bkv = np.asarray(bkv, dtype=np.float32)
    Wp = np.asarray(Wp, dtype=np.float32)
    bp = np.asarray(bp, dtype=np.float32)

    in_maps = []
    for core in range(NCORES):
        b, hp = divmod(core, 4)
        sl = slice(hp * 128, (hp + 1) * 128)
        vsl = slice(C + hp * 128, C + (hp + 1) * 128)
        m = {
            "x": np.ascontiguousarray(x[b].reshape(C, l).astype(np.float16)),
            "wq": np.ascontiguousarray(Wq[sl, :].T.astype(np.float16)),
            "bq": np.ascontiguousarray(bq[sl].reshape(128, 1)),
            "wk": np.ascontiguousarray(Wkv[sl, :].T.astype(np.float16)),
            "bk": np.ascontiguousarray(bkv[sl].reshape(128, 1)),
            "wv": np.ascontiguousarray(Wkv[vsl, :].T.astype(np.float16)),
            "bv": np.ascontiguousarray(bkv[vsl].reshape(128, 1)),
            "wp": np.ascontiguousarray(Wp[:, sl].T.astype(np.float16)),
            "bp": np.ascontiguousarray(
                (bp if hp == 0 else np.zeros_like(bp)).reshape(KT, 128).T),
        }
        in_maps.append(m)
    return in_maps


def _run(in_maps, l=L, trace=False):
    from concourse.bass_utils import run_bass_kernel_spmd
    nc = _get_nc(l)
    return run_bass_kernel_spmd(nc, in_maps, core_ids=list(range(NCORES)),
                                trace=trace)


def kernel(x, Wq, bq, Wkv, bkv, Wp, bp):
    in_maps = _shard_inputs(x, Wq, bq, Wkv, bkv, Wp, bp)
    res = _run(in_maps)
    outs = [res.results[i]["out"] for i in range(NCORES)]
    y = np.stack([outs[0] + outs[1] + outs[2] + outs[3],
                  outs[4] + outs[5] + outs[6] + outs[7]])
    return np.ascontiguousarray(y.reshape(B, C, HH, WW), dtype=np.float32)
